# revision 28
# baseline (speedup 1.0000x reference)
"""MultiHeadAttention kernel for 8x TRN2 NeuronCores.

The reference module's einsum reduces the attention tensor over BOTH the
query and key axes (attn_mass = sum_{q,k} softmax(logits)_k), and softmax
rows sum to 1, so attn_mass == Lq exactly for every (batch, head). The
whole computation therefore collapses to

    out = (Lq * (V_heads @ Wv^T + bv)).reshape(N, L, E) @ Wo^T + bo

which is a single dense GEMM after folding the (block-diagonal) per-head
V-projection into the output projection:

    out = V_flat @ W_eff + b_eff
    W_eff[h*hd+a, n] = Lq * sum_b Wv[b, a] * Wo[n, h*hd+b]      (1024 x 1024)
    b_eff[n]         = Lq * sum_{h,b} Wo[n, h*hd+b] * bv[b] + bo[n]

The device kernel is the GEMM in bf16 (the correctness gate is 2e-2
rel-err; bf16 lands ~2.6e-3), row-sharded across 8 cores (512 rows per
core), computed in TRANSPOSED orientation: out^T[n, m] = sum_k W[k, n]
X[m, k].  Each PSUM bank j holds output columns j*128..(j+1)*128 on
partitions x all 512 rows on the free dim, accumulating lhsT = W-block j
(natural layout) against rhs = X^T k-slabs.

Schedule (from NTFF trace analysis):
  * ~6.6us fixed multi-core BSP/framework preamble before any user DMA;
  * each HWDGE queue sustains only ~140-180 GB/s, so the 4.4 MB/core of
    bf16 traffic is spread over FOUR queues: X slabs alternate between
    the scalar and sync queues, W blocks split between the gpsimd and
    vector queues, outputs ride the sync queue after its X slabs drain;
  * W-block 0 is DMA'd in four quarter tiles so bank 0's first matmul
    only waits for a 64 KB transfer;
  * a few short bf16 warm-up matmuls (memset data) bridge the gap from
    the preamble barrier to first-data, lifting the PE HAM clock gate
    while spending almost none of the HAM activity budget (the HAM
    throttles the PE to ~50% duty after a sustained full-rate burst);
  * bias varies along PARTITIONS -> folded into the PSUM eviction as a
    per-partition tensor_scalar_add on the vector engine, which also
    casts fp32 PSUM to the bf16 output.

The host transposes V-shards in, and the (E, RPC) per-core outputs back.
"""

import numpy as np
import ml_dtypes

import concourse.bass as bass
import concourse.bacc as bacc
import concourse.mybir as mybir
from concourse.tile import TileContext
from concourse.bass_utils import run_bass_kernel_spmd

N_CORES = 8
E = 1024            # embed dim == d_model
H, HD = 16, 64      # heads, head dim
ROWS = 4096         # N * L = 2 * 2048
RPC = ROWS // N_CORES   # rows per core = 512
P = 128             # SBUF partitions
KT = E // P         # 8 contraction slabs
JT = E // P         # 8 output-column banks
N_WARM = 41         # short bf16 warm-up matmuls bridging preamble -> first data

_NC_CACHE = {}
LAST_RESULTS = None  # BassKernelResults of the most recent device run


def _build(dtype, n_warm=N_WARM):
    f32 = mybir.dt.float32
    odt = f32 if dtype == mybir.dt.float32r else dtype
    nc = bacc.Bacc(None, target_bir_lowering=False)
    # X^T packed as a 5-slab + 3-slab chunk -> fat DMA lines (>=2 KiB
    # lines run ~300+ GB/s/queue once the DMA path has ramped; 1 KiB
    # slab lines only ~110 GB/s).
    xa = nc.declare_dram_parameter("xa", [P, 5 * RPC], dtype, isOutput=False)
    xb = nc.declare_dram_parameter("xb", [P, 3 * RPC], dtype, isOutput=False)
    ws = [
        nc.declare_dram_parameter(f"w{j}", [P, E], dtype, isOutput=False)
        for j in range(JT)
    ]
    bw = nc.declare_dram_parameter("bw", [P, JT], f32, isOutput=False)
    outT = nc.declare_dram_parameter("outT", [E, RPC], odt, isOutput=True)

    with TileContext(nc) as tc:
        with (
            tc.tile_pool(name="xp", bufs=1) as xp,
            tc.tile_pool(name="wp", bufs=1) as wp,
            tc.tile_pool(name="bp", bufs=1) as bp,
            tc.tile_pool(name="pp", bufs=1, space="PSUM") as pp,
            tc.tile_pool(name="op", bufs=1) as op,
        ):
            # memset needs no DMA: warm-up matmuls can start right after
            # the BSP preamble, well before any input data lands.
            wm_t = bp.tile([P, P], dtype, name="wm", tag="wm")
            nc.gpsimd.memset(wm_t[:], 1.0)
            bw_t = bp.tile([P, JT], f32, name="bw", tag="bw")

            xa_t = xp.tile([P, 5 * RPC], dtype, name="xa", tag="xa")
            xb_t = xp.tile([P, 3 * RPC], dtype, name="xb", tag="xb")
            wts = [
                wp.tile([P, E], dtype, name=f"w{j}", tag=f"w{j}")
                for j in range(JT)
            ]

            # Queue plan.  Measured behavior: the DMA path ramps over the
            # first ~3us (both HW queues ~100-250 B/ns early, 300-390
            # after); first packets ~1.5-2.5us after issue; each
            # dma_start costs ~0.6-0.8us of issue time on its engine.
            # gpsimd's software-DGE queue STRANGLES the HW queues while
            # active (measured 3x collapse) -- never use it alongside
            # the critical stream.  Zero-stall feed for bank-major PE
            # emission, T0 ~= 11.9 gated by xa; w0 rides the early sync
            # ramp ahead of it, xb leads scalar:
            #   sync:   w0, xa(slabs0-4), w1, w3, w4   (+ odd outputs)
            #   scalar: xb(slabs5-7), bw, w2, w5, w6, w7 (+ even outputs)
            nc.sync.dma_start(out=wts[0][:], in_=ws[0][:, :])
            nc.sync.dma_start(out=xa_t[:], in_=xa[:, :])
            nc.scalar.dma_start(out=xb_t[:], in_=xb[:, :])
            nc.scalar.dma_start(out=bw_t[:], in_=bw[:, :])
            nc.sync.dma_start(out=wts[1][:], in_=ws[1][:, :])
            nc.scalar.dma_start(out=wts[2][:], in_=ws[2][:, :])
            nc.sync.dma_start(out=wts[3][:], in_=ws[3][:, :])
            nc.sync.dma_start(out=wts[4][:], in_=ws[4][:, :])
            for j in (5, 6, 7):
                nc.scalar.dma_start(out=wts[j][:], in_=ws[j][:, :])

            ps = [
                pp.tile([P, RPC], f32, name=f"ps{j}", tag=f"ps{j}")
                for j in range(JT)
            ]

            # Low-activity bf16 PE warm-up on nonzero memset data,
            # bridging the preamble -> first-data gap so the PE stays
            # continuously busy (p-state ramps to 2.4 GHz only after
            # ~3us of uninterrupted activity; any stall resets it).
            for i in range(n_warm):
                nc.tensor.matmul(
                    ps[i % JT][:, 0:P],
                    wm_t[:, :],
                    wm_t[:, :],
                    start=True,
                    stop=True,
                )

            def rhs(k):
                if k < 5:
                    return xa_t[:, k * RPC:(k + 1) * RPC]
                return xb_t[:, (k - 5) * RPC:(k - 4) * RPC]

            # Bank-major emission: bank j is gated by its own W block
            # (X has fully landed by then), so banks finish ~evenly
            # spread and their output DMAs overlap the tail.  The final
            # bank is split into column halves at the MATMUL level so
            # its first half's eviction + output DMA overlap the second
            # half's matmuls, shortening the kernel tail.
            hh = RPC // 2
            for j in range(JT):
                if j < JT - 1:
                    for k in range(KT):
                        nc.tensor.matmul(
                            ps[j],
                            wts[j][:, k * P:(k + 1) * P],
                            rhs(k),
                            start=(k == 0),
                            stop=(k == KT - 1),
                        )
                    o = op.tile([P, RPC], odt, name=f"o{j}", tag=f"o{j}")
                    oq = nc.scalar if j % 2 == 0 else nc.sync
                    nc.vector.tensor_scalar_add(o[:], ps[j], bw_t[:, j:j + 1])
                    oq.dma_start(out=outT[j * P:(j + 1) * P, :], in_=o[:])
                else:
                    o = op.tile([P, RPC], odt, name=f"o{j}", tag=f"o{j}")
                    # 3/4 + 1/4 column split: the big first chunk's
                    # eviction + output overlap the small second chunk's
                    # matmuls, minimizing the post-PE tail.
                    q3 = 3 * RPC // 4
                    for (lo, hi), oq in (((0, q3), nc.sync),
                                         ((q3, RPC), nc.scalar)):
                        for k in range(KT):
                            nc.tensor.matmul(
                                ps[j][:, lo:hi],
                                wts[j][:, k * P:(k + 1) * P],
                                rhs(k)[:, lo:hi],
                                start=(k == 0),
                                stop=(k == KT - 1),
                            )
                        nc.vector.tensor_scalar_add(
                            o[:, lo:hi],
                            ps[j][:, lo:hi],
                            bw_t[:, j:j + 1],
                        )
                        oq.dma_start(
                            out=outT[j * P:(j + 1) * P, lo:hi],
                            in_=o[:, lo:hi],
                        )
    nc.compile()
    return nc


def _get_nc(dtype_name, n_warm=N_WARM):
    key = (dtype_name, n_warm)
    if key not in _NC_CACHE:
        _NC_CACHE[key] = _build(getattr(mybir.dt, dtype_name), n_warm)
    return _NC_CACHE[key]


def _prep_in_maps(V, Wv, bv, Wo, bo, lq, np_dtype):
    V = np.ascontiguousarray(np.asarray(V, dtype=np.float32))
    Wv64 = np.asarray(Wv, np.float64)
    Wo64 = np.asarray(Wo, np.float64)
    bv64 = np.asarray(bv, np.float64)
    bo64 = np.asarray(bo, np.float64)

    # Fold per-head V-projection + output projection + attention mass (== Lq).
    Wo_r = Wo64.reshape(E, H, HD)                       # [n, h, b]
    W_eff = lq * np.einsum("ba,nhb->han", Wv64, Wo_r, optimize=True)
    W_eff = W_eff.reshape(E, E).astype(np.float32)      # [k, n]
    b_eff = (lq * np.einsum("nhb,b->n", Wo_r, bv64) + bo64).astype(np.float32)

    # wc[j*P + p, k*P + c] = W_eff[k*P + p, j*P + c]  (lhsT blocks, natural)
    wc = np.ascontiguousarray(
        W_eff.reshape(KT, P, JT, P).transpose(2, 1, 0, 3).reshape(JT * P, E)
    ).astype(np_dtype)
    bw_blk = np.ascontiguousarray(b_eff.reshape(JT, P).T)   # [p, j]

    wmap = {
        f"w{j}": np.ascontiguousarray(wc[j * P:(j + 1) * P, :])
        for j in range(JT)
    }
    wmap["bw"] = bw_blk

    X = V.reshape(ROWS, E).astype(np_dtype)
    in_maps = []
    for i in range(N_CORES):
        xs_i = X[i * RPC:(i + 1) * RPC, :].T.reshape(KT, P, RPC)
        m = dict(wmap)
        # chunk layout: xa[p, c*RPC + f] = X^T[c*128 + p, f]  (slabs 0-4)
        m["xa"] = np.ascontiguousarray(
            xs_i[0:5].transpose(1, 0, 2).reshape(P, 5 * RPC)
        )
        m["xb"] = np.ascontiguousarray(
            xs_i[5:8].transpose(1, 0, 2).reshape(P, 3 * RPC)
        )
        in_maps.append(m)
    return in_maps


def kernel(Q, K, V, Wq, bq, Wk, bk, Wv, bv, Wo, bo, dtype_name="bfloat16",
           n_warm=N_WARM, **_unused):
    global LAST_RESULTS
    n, L, e = np.asarray(V).shape
    lq = float(np.asarray(Q).shape[1])
    np_dtype = (np.dtype(ml_dtypes.bfloat16) if dtype_name == "bfloat16"
                else np.float32)
    in_maps = _prep_in_maps(V, Wv, bv, Wo, bo, lq, np_dtype)
    nc = _get_nc(dtype_name, n_warm)
    LAST_RESULTS = run_bass_kernel_spmd(nc, in_maps, list(range(N_CORES)))
    out = np.concatenate(
        [LAST_RESULTS.results[i]["outT"].T.astype(np.float32)
         for i in range(N_CORES)],
        axis=0,
    )
    return np.ascontiguousarray(out).reshape(n, L, E)


# revision 35
# speedup vs baseline: 1.0008x; 1.0008x over previous
"""MultiHeadAttention kernel for 8x TRN2 NeuronCores.

The reference module's einsum reduces the attention tensor over BOTH the
query and key axes (attn_mass = sum_{q,k} softmax(logits)_k), and softmax
rows sum to 1, so attn_mass == Lq exactly for every (batch, head). The
whole computation therefore collapses to

    out = (Lq * (V_heads @ Wv^T + bv)).reshape(N, L, E) @ Wo^T + bo

which is a single dense GEMM after folding the (block-diagonal) per-head
V-projection into the output projection:

    out = V_flat @ W_eff + b_eff
    W_eff[h*hd+a, n] = Lq * sum_b Wv[b, a] * Wo[n, h*hd+b]      (1024 x 1024)
    b_eff[n]         = Lq * sum_{h,b} Wo[n, h*hd+b] * bv[b] + bo[n]

The device kernel is the GEMM in bf16 (the correctness gate is 2e-2
rel-err; bf16 lands ~2.6e-3), row-sharded across 8 cores (512 rows per
core), computed in TRANSPOSED orientation: out^T[n, m] = sum_k W[k, n]
X[m, k].  Each PSUM bank j holds output columns j*128..(j+1)*128 on
partitions x all 512 rows on the free dim, accumulating lhsT = W-block j
(natural layout) against rhs = X^T k-slabs.

Schedule (from NTFF trace analysis):
  * ~6.6us fixed multi-core BSP/framework preamble before any user DMA;
  * each HWDGE queue sustains only ~140-180 GB/s, so the 4.4 MB/core of
    bf16 traffic is spread over FOUR queues: X slabs alternate between
    the scalar and sync queues, W blocks split between the gpsimd and
    vector queues, outputs ride the sync queue after its X slabs drain;
  * W-block 0 is DMA'd in four quarter tiles so bank 0's first matmul
    only waits for a 64 KB transfer;
  * a few short bf16 warm-up matmuls (memset data) bridge the gap from
    the preamble barrier to first-data, lifting the PE HAM clock gate
    while spending almost none of the HAM activity budget (the HAM
    throttles the PE to ~50% duty after a sustained full-rate burst);
  * bias varies along PARTITIONS -> folded into the PSUM eviction as a
    per-partition tensor_scalar_add on the vector engine, which also
    casts fp32 PSUM to the bf16 output.

The host transposes V-shards in, and the (E, RPC) per-core outputs back.
"""

import numpy as np
import ml_dtypes

import concourse.bass as bass
import concourse.bacc as bacc
import concourse.mybir as mybir
from concourse.tile import TileContext
from concourse.bass_utils import run_bass_kernel_spmd

N_CORES = 8
E = 1024            # embed dim == d_model
H, HD = 16, 64      # heads, head dim
ROWS = 4096         # N * L = 2 * 2048
RPC = ROWS // N_CORES   # rows per core = 512
P = 128             # SBUF partitions
KT = E // P         # 8 contraction slabs
JT = E // P         # 8 output-column banks
N_WARM = 36         # short bf16 warm-up matmuls bridging preamble -> first data

_NC_CACHE = {}
LAST_RESULTS = None  # BassKernelResults of the most recent device run


def _build(dtype, n_warm=N_WARM):
    f32 = mybir.dt.float32
    odt = f32 if dtype == mybir.dt.float32r else dtype
    nc = bacc.Bacc(None, target_bir_lowering=False)
    # X^T packed as two 4-slab chunks -> fat DMA lines (>=2 KiB lines
    # run ~300+ GB/s/queue once the DMA path has ramped; 1 KiB slab
    # lines only ~110 GB/s).
    xa = nc.declare_dram_parameter("xa", [P, 4 * RPC], dtype, isOutput=False)
    xb = nc.declare_dram_parameter("xb", [P, 4 * RPC], dtype, isOutput=False)
    ws = [
        nc.declare_dram_parameter(f"w{j}", [P, E], dtype, isOutput=False)
        for j in range(JT)
    ]
    bw = nc.declare_dram_parameter("bw", [P, JT], f32, isOutput=False)
    outT = nc.declare_dram_parameter("outT", [E, RPC], odt, isOutput=True)

    with TileContext(nc) as tc:
        with (
            tc.tile_pool(name="xp", bufs=1) as xp,
            tc.tile_pool(name="wp", bufs=1) as wp,
            tc.tile_pool(name="bp", bufs=1) as bp,
            tc.tile_pool(name="pp", bufs=1, space="PSUM") as pp,
            tc.tile_pool(name="op", bufs=1) as op,
        ):
            # memset needs no DMA: warm-up matmuls can start right after
            # the BSP preamble, well before any input data lands.
            wm_t = bp.tile([P, P], dtype, name="wm", tag="wm")
            nc.gpsimd.memset(wm_t[:], 1.0)
            bw_t = bp.tile([P, JT], f32, name="bw", tag="bw")

            xa_t = xp.tile([P, 4 * RPC], dtype, name="xa", tag="xa")
            xb_t = xp.tile([P, 4 * RPC], dtype, name="xb", tag="xb")
            wts = [
                wp.tile([P, E], dtype, name=f"w{j}", tag=f"w{j}")
                for j in range(JT)
            ]

            # Queue plan.  Measured behavior: the DMA path ramps over the
            # first ~3us (both HW queues ~100-250 B/ns early, 300-390
            # after); first packets ~1.5-2.5us after issue; each
            # dma_start costs ~0.6-0.8us of issue time on its engine.
            # gpsimd's software-DGE queue STRANGLES the HW queues while
            # active (measured 3x collapse) -- never use it alongside
            # the critical stream.  Zero-stall feed for bank-major PE
            # emission (bank0 k0-3 first), T0 ~= 11.2 gated by w0+xa
            # (0.75 MB); w0 rides the early sync ramp, xb leads scalar:
            #   sync:   w0, xa(slabs0-3), w1, w3, w4   (+ odd outputs)
            #   scalar: xb(slabs4-7), bw, w2, w5, w6, w7 (+ even outputs)
            nc.sync.dma_start(out=wts[0][:], in_=ws[0][:, :])
            nc.sync.dma_start(out=xa_t[:], in_=xa[:, :])
            nc.scalar.dma_start(out=xb_t[:], in_=xb[:, :])
            nc.scalar.dma_start(out=bw_t[:], in_=bw[:, :])
            nc.sync.dma_start(out=wts[1][:], in_=ws[1][:, :])
            nc.scalar.dma_start(out=wts[2][:], in_=ws[2][:, :])
            nc.sync.dma_start(out=wts[3][:], in_=ws[3][:, :])
            nc.sync.dma_start(out=wts[4][:], in_=ws[4][:, :])
            for j in (5, 6, 7):
                nc.scalar.dma_start(out=wts[j][:], in_=ws[j][:, :])

            ps = [
                pp.tile([P, RPC], f32, name=f"ps{j}", tag=f"ps{j}")
                for j in range(JT)
            ]

            # Low-activity bf16 PE warm-up on nonzero memset data,
            # bridging the preamble -> first-data gap so the PE stays
            # continuously busy (p-state ramps to 2.4 GHz only after
            # ~3us of uninterrupted activity; any stall resets it).
            for i in range(n_warm):
                nc.tensor.matmul(
                    ps[i % JT][:, 0:P],
                    wm_t[:, :],
                    wm_t[:, :],
                    start=True,
                    stop=True,
                )

            def rhs(k):
                t = xa_t if k < 4 else xb_t
                return t[:, (k % 4) * RPC:(k % 4 + 1) * RPC]

            # Emission order: bank 0's k0-3 partial accumulation first
            # (its gate is only w0 + xa = 0.75 MB), then banks 1-7 in
            # full (X has fully landed by bank 1), then bank 0's k4-7
            # finish.  Banks therefore complete ~evenly spread and
            # their output DMAs overlap the tail; the LAST eviction is
            # split 3/4 + 1/4 with SEPARATE o tiles (a shared tile
            # makes Tile serialize the second eviction behind the
            # first chunk's output DMA delivery -- whole-tile WAR).
            for k in range(4):
                nc.tensor.matmul(
                    ps[0],
                    wts[0][:, k * P:(k + 1) * P],
                    rhs(k),
                    start=(k == 0),
                    stop=False,
                )
            for j in range(1, JT):
                for k in range(KT):
                    nc.tensor.matmul(
                        ps[j],
                        wts[j][:, k * P:(k + 1) * P],
                        rhs(k),
                        start=(k == 0),
                        stop=(k == KT - 1),
                    )
                o = op.tile([P, RPC], odt, name=f"o{j}", tag=f"o{j}")
                oq = nc.scalar if j % 2 == 0 else nc.sync
                nc.vector.tensor_scalar_add(o[:], ps[j], bw_t[:, j:j + 1])
                oq.dma_start(out=outT[j * P:(j + 1) * P, :], in_=o[:])
            for k in range(4, KT):
                nc.tensor.matmul(
                    ps[0],
                    wts[0][:, k * P:(k + 1) * P],
                    rhs(k),
                    start=False,
                    stop=(k == KT - 1),
                )
            q3 = 3 * RPC // 4
            for (lo, hi), oq, tag in (((0, q3), nc.sync, "o0a"),
                                      ((q3, RPC), nc.scalar, "o0b")):
                o = op.tile([P, hi - lo], odt, name=tag, tag=tag)
                nc.vector.tensor_scalar_add(o[:], ps[0][:, lo:hi], bw_t[:, 0:1])
                oq.dma_start(out=outT[0:P, lo:hi], in_=o[:])
    nc.compile()
    return nc


def _get_nc(dtype_name, n_warm=N_WARM):
    key = (dtype_name, n_warm)
    if key not in _NC_CACHE:
        _NC_CACHE[key] = _build(getattr(mybir.dt, dtype_name), n_warm)
    return _NC_CACHE[key]


def _prep_in_maps(V, Wv, bv, Wo, bo, lq, np_dtype):
    V = np.ascontiguousarray(np.asarray(V, dtype=np.float32))
    Wv64 = np.asarray(Wv, np.float64)
    Wo64 = np.asarray(Wo, np.float64)
    bv64 = np.asarray(bv, np.float64)
    bo64 = np.asarray(bo, np.float64)

    # Fold per-head V-projection + output projection + attention mass (== Lq).
    Wo_r = Wo64.reshape(E, H, HD)                       # [n, h, b]
    W_eff = lq * np.einsum("ba,nhb->han", Wv64, Wo_r, optimize=True)
    W_eff = W_eff.reshape(E, E).astype(np.float32)      # [k, n]
    b_eff = (lq * np.einsum("nhb,b->n", Wo_r, bv64) + bo64).astype(np.float32)

    # wc[j*P + p, k*P + c] = W_eff[k*P + p, j*P + c]  (lhsT blocks, natural)
    wc = np.ascontiguousarray(
        W_eff.reshape(KT, P, JT, P).transpose(2, 1, 0, 3).reshape(JT * P, E)
    ).astype(np_dtype)
    bw_blk = np.ascontiguousarray(b_eff.reshape(JT, P).T)   # [p, j]

    wmap = {
        f"w{j}": np.ascontiguousarray(wc[j * P:(j + 1) * P, :])
        for j in range(JT)
    }
    wmap["bw"] = bw_blk

    X = V.reshape(ROWS, E).astype(np_dtype)
    in_maps = []
    for i in range(N_CORES):
        xs_i = X[i * RPC:(i + 1) * RPC, :].T.reshape(KT, P, RPC)
        m = dict(wmap)
        # chunk layout: xa[p, c*RPC + f] = X^T[c*128 + p, f]  (slabs 0-3)
        m["xa"] = np.ascontiguousarray(
            xs_i[0:4].transpose(1, 0, 2).reshape(P, 4 * RPC)
        )
        m["xb"] = np.ascontiguousarray(
            xs_i[4:8].transpose(1, 0, 2).reshape(P, 4 * RPC)
        )
        in_maps.append(m)
    return in_maps


def kernel(Q, K, V, Wq, bq, Wk, bk, Wv, bv, Wo, bo, dtype_name="bfloat16",
           n_warm=N_WARM, **_unused):
    global LAST_RESULTS
    n, L, e = np.asarray(V).shape
    lq = float(np.asarray(Q).shape[1])
    np_dtype = (np.dtype(ml_dtypes.bfloat16) if dtype_name == "bfloat16"
                else np.float32)
    in_maps = _prep_in_maps(V, Wv, bv, Wo, bo, lq, np_dtype)
    nc = _get_nc(dtype_name, n_warm)
    LAST_RESULTS = run_bass_kernel_spmd(nc, in_maps, list(range(N_CORES)))
    out = np.concatenate(
        [LAST_RESULTS.results[i]["outT"].T.astype(np.float32)
         for i in range(N_CORES)],
        axis=0,
    )
    return np.ascontiguousarray(out).reshape(n, L, E)
